# revision 9
# baseline (speedup 1.0000x reference)
"""BinConv2d (XNOR-Net style) Trainium2 kernel, 8-core data-parallel, v4.

Layer math (BatchNorm train-mode -> BinActiv -> binary 3x3 conv -> scale by
box-filtered channel-mean magnitudes and per-filter alpha -> relu):

  mu, var: batch stats of x over (N, H, W) per channel      (needs all-reduce)
  xn  = (x - mu) * rsqrt(var + eps) * gamma + beta
  m   = mean_c |xn|;  xb = sign(xn);  Wb = sign(W);  alpha = mean |W| per filter
  y   = conv(xb, Wb, pad=1) + b
  out = relu(y * box3x3(m) * alpha)

v4 structure. The stats-landing time is floored by ncfw one-time setup
(~52us from the warmup trigger) + warmup mesh + real mesh ~= 72-76us, so the
only things that matter are (a) everything that does NOT depend on global
stats must be done by then, and (b) the land->conv-start chain must be
minimal.  v3 lost ~36us here: its ACT queue was still draining m-path ABS
work when the stats landed, and the global scalar chain was stuck behind
m-path copies in the DVE queue, so the first conv matmul issued ~36us after
the collective.

  pass 1:  load x (bf16, host-converted) into resident SBUF.  ALL BN stats
           on DVE bn_stats (ACT freed entirely); trigger lands ~45us which
           is still well before ncfw is ready, so it costs nothing.
  window:  weights arrive host-pre-transposed into the DoubleRow lhsT layout
           [ki, tap, ko, co] (pure host permutation), so weight prep is ONE
           ACT sign -> fp8 and ONE ACT abs -> fp8 (alpha via 36 tiny N=1
           PE matmuls against ones), no PE transposes, no per-tile signs.
           m path with LOCAL per-core stats as in v3 (ACT abs -> fp8 |xn|,
           ones-matmul per row chunk, box3x3, gpsimd broadcast).  ACT order:
           abs(kc0 imgs), weight sign/abs, abs(kc1 imgs) -> ACT idle from
           ~61us, waiting on the collective.
  land:    readback -> global scalar chain (DVE ops emitted after all
           window DVE work so they run immediately) -> sign img0 split in
           row slabs so conv group A starts after the first two slabs.
           Two PE warm-up chains (gated on the readback / on t') keep the
           PE HAM-hot through the land window so conv issues at 2.4 GHz.
  pass 2:  conv via 9 shifted DoubleRow matmuls per chunk group (PSUM 4+3
           banks), fused relu(scale,bias) on ACT, abeta multiply on DVE,
           bf16 DMA out.  Image 3 ends with a 1-chunk group to cut the tail.

  sign needs *global* mu (local-stats threshold flips ~0.24% of pixels =>
  ~10% output error), so the conv cannot start before the collective lands;
  the m path tolerates local stats.
"""

import os
import sys

import numpy as np

for _p in ("/opt/trn_rl_repo", "/root/.axon_site/_ro/trn_rl_repo"):
    if os.path.isdir(_p) and _p not in sys.path:
        sys.path.insert(0, _p)

import concourse.bass as bass  # noqa: E402
import concourse.bacc as bacc  # noqa: E402
import concourse.mybir as mybir  # noqa: E402
import concourse.tile as tile  # noqa: E402
from concourse.bass_utils import run_bass_kernel_spmd  # noqa: E402

F32 = mybir.dt.float32
BF16 = mybir.dt.bfloat16
FP8 = mybir.dt.float8e4
NPBF16 = mybir.dt.np(BF16)
AF = mybir.ActivationFunctionType
ALU = mybir.AluOpType
AX = mybir.AxisListType

EPS = 1e-4
NCORES = 8
P = 128
CIN = 256
COUT = 256
H = 56
W = 56
HP = H + 2          # 58 padded rows
WP = W + 2          # 58 padded cols
IMGP = HP * WP      # 3364 padded pixels / image
NPIX = H * W        # 3136 true pixels / image
MARGIN = 64         # dead zero margin absorbing out-of-image tap reads
CH_ROWS = 8         # output rows per PSUM bank
NCH = H // CH_ROWS  # 7 chunks / image
CF = CH_ROWS * W    # 448 compact free elems / chunk
CFP = CH_ROWS * WP  # 464 padded free elems / chunk
BANK = 512          # f32 elems per PSUM bank
KTAPS = 9
WARMUP = os.environ.get("BC_WARMUP", "1") == "1"


def _build(n_local: int):
    NL = n_local

    nc = bacc.Bacc("TRN2", debug=False, target_bir_lowering=False,
                   num_devices=NCORES)
    x_d = nc.declare_dram_parameter("x", [NL, CIN, H, W], BF16, isOutput=False)
    g_d = nc.declare_dram_parameter("gamma", [CIN], F32, isOutput=False)
    bb_d = nc.declare_dram_parameter("beta_bn", [CIN], F32, isOutput=False)
    # host-pre-permuted weights: [ki, tap, ko, co] with c = ko*128 + ki
    wt_d = nc.declare_dram_parameter("Wt", [P, KTAPS, 2, COUT], BF16,
                                     isOutput=False)
    b_d = nc.declare_dram_parameter("b", [COUT], F32, isOutput=False)
    id_d = nc.declare_dram_parameter("ident", [P, P], BF16, isOutput=False)
    tv_d = nc.declare_dram_parameter("tvt", [HP, H], BF16, isOutput=False)
    out_d = nc.declare_dram_parameter("out", [NL, COUT, H, W], BF16, isOutput=True)

    with tile.TileContext(nc, num_cores=NCORES) as tc:
        with (
            tc.tile_pool(name="statics", bufs=1) as st,
            tc.tile_pool(name="axp", bufs=3) as axp,
            tc.tile_pool(name="smalls", bufs=2) as sm,
            tc.tile_pool(name="zp", bufs=2) as zp,
            tc.tile_pool(name="outp", bufs=3) as outp,
            tc.tile_pool(name="psA", bufs=1, space="PSUM") as psA,
            tc.tile_pool(name="psB", bufs=1, space="PSUM") as psB,
            tc.tile_pool(name="psS", bufs=1, space="PSUM") as psS,
            tc.tile_pool(name="dram", bufs=1, space="DRAM") as dr,
        ):
            # ---------------- warmup collective (very first gpsimd op) -----
            # ncfw pays ~52us of one-time setup on the first collective; fire
            # a throwaway AllGather with no data deps immediately so the real
            # one only pays mesh latency.
            if WARMUP:
                wu_in = dr.tile([1, 8], F32, name="wu_in", tag="wu_in")
                wu_out = dr.tile([NCORES, 1, 8], F32, name="wu_out",
                                 tag="wu_out", addr_space="Shared")
                nc.gpsimd.collective_compute(
                    "AllGather", ALU.bypass,
                    replica_groups=[list(range(NCORES))],
                    ins=[wu_in.opt()], outs=[wu_out.opt()],
                )

            # ---------------- pass 1: load x + BN statistics (all DVE) -----
            xr = st.tile([P, 2 * NL * NPIX], BF16, name="xr", tag="xr")
            xrv = xr.rearrange("p (k i f) -> p k i f", k=2, i=NL)
            stats = []
            for kc in range(2):
                sb = st.tile([P, NL * NCH * 6], F32, name=f"stats{kc}",
                             tag=f"stats{kc}")
                stats.append(sb)
            for img in range(NL):
                for kc in range(2):
                    deng = nc.sync if kc == 0 else nc.scalar
                    deng.dma_start(
                        xrv[:, kc, img, :],
                        x_d.ap()[img, kc * P:(kc + 1) * P]
                        .rearrange("c h w -> c (h w)"),
                    )
            mean_h, var_h = [None, None], [None, None]
            pay = st.tile([P, 4], F32, name="pay", tag="pay")
            epsc = st.tile([P, 1], F32, name="epsc", tag="epsc")
            nc.vector.memset(epsc[:], EPS)
            ones2 = st.tile([P, 2], FP8, name="ones2", tag="ones2")
            nc.vector.memset(ones2[:], 1.0)

            # host constants + weights (scalar=HWDGE ring shared with kc1)
            ident = st.tile([P, P], BF16, name="ident_sb", tag="ident_sb")
            nc.scalar.dma_start(ident[:], id_d.ap())
            tvt = st.tile([HP, H], BF16, name="tvt_sb", tag="tvt_sb")
            nc.scalar.dma_start(tvt[:], tv_d.ap())
            gam2 = st.tile([P, 2], F32, name="gam2", tag="gam2")
            nc.scalar.dma_start(gam2[:], g_d.ap().rearrange("(k p) -> p k", k=2))
            bet2 = st.tile([P, 2], F32, name="bet2", tag="bet2")
            nc.scalar.dma_start(bet2[:], bb_d.ap().rearrange("(k p) -> p k", k=2))
            bvec2 = st.tile([P, 2], F32, name="bvec2", tag="bvec2")
            nc.scalar.dma_start(bvec2[:], b_d.ap().rearrange("(k p) -> p k", k=2))
            w_t = st.tile([P, KTAPS * 2 * COUT], BF16, name="w_t", tag="w_t")
            nc.sync.dma_start(w_t[:],
                              wt_d.ap().rearrange("p t k o -> p (t k o)"))

            ACT_STATS = []
            acc_s, acc_q = {}, {}

            s_loc, bstar = [None, None], [None, None]

            def kc_stats(kc):
                dve_imgs = [i for i in range(NL) if (kc, i) not in ACT_STATS]
                for gi, img in enumerate(dve_imgs):
                    for g in range(NCH):
                        col = (gi * NCH + g) * 6
                        nc.vector.bn_stats(
                            stats[kc][:, col:col + 6],
                            xrv[:, kc, img, g * CF:(g + 1) * CF],
                        )
                nd = len(dve_imgs)
                agg = st.tile([P, 2], F32, name=f"agg{kc}", tag=f"agg{kc}")
                nc.vector.bn_aggr(agg[:], stats[kc][:, 0:nd * NCH * 6])
                msq = sm.tile([P, 1], F32, name="msq", tag="msq")
                nc.vector.tensor_mul(msq[:], agg[:, 0:1], agg[:, 0:1])
                ex2 = st.tile([P, 1], F32, name=f"ex2_{kc}", tag=f"ex2_{kc}")
                nc.vector.tensor_add(ex2[:], agg[:, 1:2], msq[:])
                if nd == NL:
                    nc.vector.tensor_copy(pay[:, 2 * kc:2 * kc + 1], agg[:, 0:1])
                    nc.vector.tensor_copy(pay[:, 2 * kc + 1:2 * kc + 2], ex2[:])
                    mean_h[kc] = agg[:, 0:1]
                    var_h[kc] = agg[:, 1:2]
                else:
                    # combine DVE aggregate (nd imgs) with ACT raw sums
                    # (equal image weights): S = nd*NPIX*mean + sum(s_i)
                    a_imgs = [i for i in range(NL) if (kc, i) in ACT_STATS]
                    Ssum = sm.tile([P, 1], F32, name="Ssum", tag="Ssum")
                    nc.vector.tensor_add(Ssum[:], acc_s[(kc, a_imgs[0])][:],
                                         acc_s[(kc, a_imgs[1])][:])
                    mn = sm.tile([P, 1], F32, name="mn", tag="mn")
                    nc.vector.tensor_scalar_mul(mn[:], agg[:, 0:1],
                                                float(nd * NPIX))
                    nc.vector.tensor_add(Ssum[:], Ssum[:], mn[:])
                    Qsum = sm.tile([P, 1], F32, name="Qsum", tag="Qsum")
                    nc.vector.tensor_add(Qsum[:], acc_q[(kc, a_imgs[0])][:],
                                         acc_q[(kc, a_imgs[1])][:])
                    en = sm.tile([P, 1], F32, name="en", tag="en")
                    nc.vector.tensor_scalar_mul(en[:], ex2[:],
                                                float(nd * NPIX))
                    nc.vector.tensor_add(Qsum[:], Qsum[:], en[:])
                    meanf = st.tile([P, 1], F32, name=f"meanf{kc}",
                                    tag=f"meanf{kc}")
                    nc.vector.tensor_scalar_mul(meanf[:], Ssum[:],
                                                1.0 / (NL * NPIX))
                    ex2f = st.tile([P, 1], F32, name=f"ex2f{kc}",
                                   tag=f"ex2f{kc}")
                    nc.vector.tensor_scalar_mul(ex2f[:], Qsum[:],
                                                1.0 / (NL * NPIX))
                    nc.vector.tensor_copy(pay[:, 2 * kc:2 * kc + 1], meanf[:])
                    nc.vector.tensor_copy(pay[:, 2 * kc + 1:2 * kc + 2],
                                          ex2f[:])
                    mnsq = sm.tile([P, 1], F32, name="mnsq", tag="mnsq")
                    nc.vector.tensor_mul(mnsq[:], meanf[:], meanf[:])
                    varf = st.tile([P, 1], F32, name=f"varf{kc}",
                                   tag=f"varf{kc}")
                    nc.vector.tensor_sub(varf[:], ex2f[:], mnsq[:])
                    mean_h[kc] = meanf[:]
                    var_h[kc] = varf[:]
                # local scalars for the m path
                sigl = sm.tile([P, 1], F32, name="sigl", tag="sigl")
                nc.scalar.activation(sigl[:], var_h[kc], AF.Sqrt, bias=epsc[:])
                rsigl = sm.tile([P, 1], F32, name="rsigl", tag="rsigl")
                nc.vector.reciprocal(rsigl[:], sigl[:])
                sl = st.tile([P, 1], F32, name=f"sl{kc}", tag=f"sl{kc}")
                nc.vector.tensor_mul(sl[:], gam2[:, kc:kc + 1], rsigl[:])
                s_loc[kc] = sl
                smu = sm.tile([P, 1], F32, name="smu", tag="smu")
                nc.vector.tensor_mul(smu[:], sl[:], mean_h[kc])
                bs = st.tile([P, 1], F32, name=f"bs{kc}", tag=f"bs{kc}")
                nc.vector.tensor_sub(bs[:], bet2[:, kc:kc + 1], smu[:])
                bstar[kc] = bs

            kc_stats(0)

            # ---------------- window: m-path ABS kc0 (ACT, right after the
            # kc0 local scalars; kc1 half comes after the weight prep) ------
            ax_t = []
            for img in range(NL):
                ax = axp.tile([P, 2 * NPIX], FP8, name="ax", tag="ax")
                ax_t.append(ax.rearrange("p (k f) -> p k f", k=2))
                nc.scalar.activation(ax_t[img][:, 0, :], xrv[:, 0, img, :],
                                     AF.Abs, bias=bstar[0][:],
                                     scale=s_loc[0][:])

            # ---------------- pass 1b: kc1 stats + trigger -----------------
            kc_stats(1)
            cc_in = dr.tile([P, 4], F32, name="cc_in", tag="cc_in")
            cc_out = dr.tile([NCORES, P, 4], F32, name="cc_out", tag="cc_out",
                             addr_space="Shared")
            nc.sync.dma_start(cc_in[:], pay[:])
            nc.gpsimd.collective_compute(
                "AllGather", ALU.bypass,
                replica_groups=[list(range(NCORES))],
                ins=[cc_in.opt()], outs=[cc_out.opt()],
            )
            # readback emitted now so it sits at the head of the (otherwise
            # idle) sync ring the moment the collective completes
            ag_sb = st.tile([P, NCORES * 4], F32, name="ag_sb", tag="ag_sb")
            nc.sync.dma_start(
                ag_sb[:].rearrange("p (r c) -> p r c", c=4),
                cc_out.rearrange("r p c -> p r c"),
            )

            # ---------------- static pads (gpsimd) -------------------------
            # xq blocks: [P, img, ko, M | IMGP | M] so the DoubleRow rhs view
            # (k-stride = BLK) bounding-boxes only ONE image's two halves.
            BLK = IMGP + 2 * MARGIN
            xq = st.tile([P, NL * 2 * BLK], FP8, name="xq", tag="xq")
            xqb = xq.rearrange("p (i k b) -> p i k b", i=NL, k=2)
            for img in range(NL):
                for ko in range(2):
                    nc.gpsimd.memset(xqb[:, img, ko, 0:MARGIN + WP], 0.0)
                    nc.gpsimd.memset(
                        xqb[:, img, ko, MARGIN + (HP - 1) * WP:BLK], 0.0)
                    colv = (xqb[:, img, ko, MARGIN + WP: MARGIN + (HP - 1) * WP]
                            .rearrange("p (h w) -> p h w", w=WP))
                    nc.gpsimd.memset(colv[:, :, 0:1], 0.0)
                    nc.gpsimd.memset(colv[:, :, WP - 1:WP], 0.0)
            m_flat = st.tile([1, NL * IMGP], BF16, name="m_flat", tag="m_flat")
            mfl = m_flat.rearrange("p (i f) -> p i f", i=NL)
            nc.gpsimd.memset(mfl[:, :, 0:WP], 0.0)
            nc.gpsimd.memset(mfl[:, :, (HP - 1) * WP:IMGP], 0.0)
            mfv = mfl[:, :, WP:(HP - 1) * WP].rearrange("p i (h w) -> p i h w",
                                                        w=WP)
            nc.gpsimd.memset(mfv[:, :, :, 0:1], 0.0)
            nc.gpsimd.memset(mfv[:, :, :, WP - 1:WP], 0.0)

            # ---------------- window: m path (kc1 abs + chunk matmuls) -----
            abeta = st.tile([P, NL * NPIX], BF16, name="abeta", tag="abeta")
            abv = abeta.rearrange("p (i f) -> p i f", i=NL)
            for img in range(NL):
                nc.scalar.activation(ax_t[img][:, 1, :], xrv[:, 1, img, :],
                                     AF.Abs, bias=bstar[1][:],
                                     scale=s_loc[1][:])
                for ch in range(NCH):
                    # ping-pong between psS and the (window-idle) conv psB
                    # bank so the per-chunk mm->copy chain pipelines
                    pool, ptag = ((psS, "s") if (img * NCH + ch) % 2 == 0
                                  else (psB, "cvB"))
                    mp = pool.tile([1, CF], F32, name="mp", tag=ptag)
                    nc.tensor.matmul(mp[:], ones2[:, 0:1],
                                     ax_t[img][:, 0, ch * CF:(ch + 1) * CF],
                                     start=True, stop=False)
                    nc.tensor.matmul(mp[:], ones2[:, 1:2],
                                     ax_t[img][:, 1, ch * CF:(ch + 1) * CF],
                                     start=False, stop=True)
                    mfi = (m_flat[:, img * IMGP:(img + 1) * IMGP]
                           .rearrange("p (h w) -> p h w", w=WP))
                    nc.vector.tensor_copy(
                        mfi[:, 1 + ch * CH_ROWS: 1 + (ch + 1) * CH_ROWS,
                            1:1 + W],
                        mp.rearrange("p (h w) -> p h w", w=W),
                    )
                # beta map: horizontal sum on DVE, vertical via banded matmul
                mhw = sm.tile([HP, WP], BF16, name="mhw", tag="mhw")
                nc.gpsimd.dma_start(mhw[:], m_flat[:, img * IMGP:(img + 1) * IMGP])
                hs = sm.tile([HP, WP], BF16, name="hs", tag="hs")
                nc.vector.tensor_add(hs[:, 1:1 + W], mhw[:, 0:W], mhw[:, 2:2 + W])
                nc.vector.tensor_add(hs[:, 1:1 + W], hs[:, 1:1 + W],
                                     mhw[:, 1:1 + W])
                bps = psS.tile([H, W], F32, name="bps", tag="s")
                nc.tensor.matmul(bps[:], tvt[:], hs[:, 1:1 + W], start=True,
                                 stop=True)
                bhw = sm.tile([H, W], BF16, name="bhw", tag="bhw")
                nc.vector.tensor_copy(bhw[:], bps[:])
                bflat = sm.tile([1, NPIX], BF16, name="bflat", tag="bflat",
                                bufs=2)
                nc.gpsimd.dma_start(bflat[:], bhw[:])
                nc.gpsimd.partition_broadcast(abv[:, img, :], bflat[:])

            # ---------------- window: weight prep (single sign + abs) ------
            wq = st.tile([P, KTAPS * 2 * COUT], FP8, name="wq", tag="wq")
            wqv = wq.rearrange("p (t k o) -> p t k o", t=KTAPS, k=2)
            nc.scalar.activation(wq[:], w_t[:], AF.Sign)
            awq = st.tile([P, KTAPS * 2 * COUT], FP8, name="awq", tag="awq")
            awqv = awq.rearrange("p (t k o) -> p t k o", t=KTAPS, k=2)
            nc.scalar.activation(awq[:], w_t[:], AF.Abs)
            # alpha[co] = mean |W[co,:,:,:]| : accumulate 18 N=1 matmuls
            alpha_sc, ab = [], []
            for oc in range(2):
                aps = psS.tile([P, 1], F32, name="aps", tag="s")
                first = True
                for tap in range(KTAPS):
                    for ko in range(2):
                        nc.tensor.matmul(
                            aps[:], awqv[:, tap, ko, oc * P:(oc + 1) * P],
                            ones2[:, 0:1],
                            start=first, stop=(tap == KTAPS - 1 and ko == 1),
                        )
                        first = False
                asc = st.tile([P, 1], F32, name=f"alph{oc}", tag=f"alph{oc}")
                nc.vector.tensor_scalar_mul(asc[:], aps[:], 1.0 / (CIN * KTAPS))
                alpha_sc.append(asc)
                abt = st.tile([P, 1], F32, name=f"ab{oc}", tag=f"ab{oc}")
                nc.vector.tensor_mul(abt[:], asc[:], bvec2[:, oc:oc + 1])
                ab.append(abt)
            rgam = st.tile([P, 2], F32, name="rgam", tag="rgam")
            nc.vector.reciprocal(rgam[:], gam2[:])
            brg = st.tile([P, 2], F32, name="brg", tag="brg")
            nc.vector.tensor_mul(brg[:], bet2[:], rgam[:])


            # ---------------- global scalars (DVE ops emitted after the
            # window DVE work, so they run the moment the readback lands) ---
            arsb = st.tile([P, 4], F32, name="arsb", tag="arsb")
            nc.vector.tensor_reduce(
                arsb[:], ag_sb[:].rearrange("p (r c) -> p c r", c=4),
                axis=AX.X, op=ALU.add,
            )
            mue = st.tile([P, 4], F32, name="mue", tag="mue")
            nc.vector.tensor_scalar_mul(mue[:], arsb[:], 1.0 / NCORES)
            muev = mue.rearrange("p (c k) -> p c k", c=2)  # [P, kc, (mean,ex2)]
            muv = muev[:, :, 0]
            musq = sm.tile([P, 2], F32, name="musq2", tag="musq2")
            nc.vector.tensor_mul(musq[:], muv, muv)
            varv = sm.tile([P, 2], F32, name="varv", tag="varv")
            nc.vector.tensor_sub(varv[:], muev[:, :, 1], musq[:])
            # PE p-state warmup: one short chain gated on the collective
            # result (first MM reads wrm; the rest chain via the psS WAR
            # dependency), dense enough to unthrottle HAM before the conv.
            # v4's 148-MM chains got interleaved into the conv by the
            # scheduler and cost ~15us of serialized queue time.
            wrm = sm.tile([P, 4], BF16, name="wrm", tag="wrm")
            nc.vector.tensor_copy(wrm[:], arsb[:])
            pd = psS.tile([P, 4], F32, name="pd", tag="s")
            nc.tensor.matmul(pd[:], ident[:], wrm[:], start=True, stop=True)
            for _ in range(8):
                pd = psS.tile([P, CF], F32, name="pd", tag="s")
                nc.tensor.matmul(pd[:], ident[:], xr[:, 0:CF], start=True,
                                 stop=True)
            sigv = sm.tile([P, 2], F32, name="sigv", tag="sigv")
            nc.scalar.activation(sigv[:], varv[:], AF.Sqrt, bias=epsc[:])
            tb2 = sm.tile([P, 2], F32, name="tb2", tag="tb2")
            nc.vector.tensor_mul(tb2[:], brg[:], sigv[:])
            tp = st.tile([P, 2], F32, name="tp", tag="tp")
            nc.vector.tensor_sub(tp[:], tb2[:], muv)
            # ---------------- pass 2: sign + conv + epilogue ----------------
            GRPS_STD = [(0, 4, psA, "cvA"), (4, 3, psB, "cvB")]
            GRPS_LAST = [(0, 3, psA, "cvA"), (3, 3, psB, "cvB"),
                         (6, 1, psS, "s")]

            def sign_img(img, slabs=False):
                # slabs: kc0 whole, then kc1 rows 0..33 (the DoubleRow rhs
                # bounding box spans the whole kc0 block, so splitting kc0
                # buys nothing; group A of the first image reads kc1 padded
                # rows <= 34), so conv starts ~1us earlier.
                rr = ([(0, 0, H), (1, 0, 34), (1, 34, H)] if slabs
                      else [(0, 0, H), (1, 0, H)])
                for kc, r0, r1 in rr:
                    xqv = (xqb[:, img, kc, MARGIN:MARGIN + IMGP]
                           .rearrange("p (h w) -> p h w", w=WP))
                    nc.scalar.activation(
                        xqv[:, 1 + r0:1 + r1, 1:1 + W],
                        xrv[:, kc, img, r0 * W:r1 * W]
                        .rearrange("p (h w) -> p h w", w=W),
                        AF.Sign, bias=tp[:, kc:kc + 1],
                    )

            def conv_img(img, mid_cb=None):
                grps = GRPS_LAST if img == NL - 1 else GRPS_STD
                for oc in range(2):
                    if oc == 1 and mid_cb is not None:
                        # queue the next image's signs on ACT *between* this
                        # image's oc halves: the oc0 relus stay ahead of the
                        # signs so the oc1 PSUM reuse never stalls on ACT
                        mid_cb()
                    for (c0, nch, pool, tag) in grps:
                        cv = pool.tile([P, nch * BANK], F32, name="cv", tag=tag)
                        for tap in range(KTAPS):
                            dh, dw = tap // 3, tap % 3
                            off = (dh - 1) * WP + (dw - 1)
                            for ch in range(nch):
                                base = (MARGIN
                                        + (1 + (c0 + ch) * CH_ROWS) * WP + off)
                                nc.tensor.matmul(
                                    cv[:, ch * BANK:ch * BANK + CFP],
                                    wqv[:, tap, :, oc * P:(oc + 1) * P],
                                    xqb[:, img, :, base: base + CFP],
                                    start=(tap == 0), stop=(tap == KTAPS - 1),
                                    perf_mode=mybir.MatmulPerfMode.DoubleRow,
                                )
                        # fused relu(alpha*cv + alpha*b) over the whole group
                        cvv = (cv.rearrange("p (c x) -> p c x", x=BANK)
                               [:, :, 0:CFP]
                               .rearrange("p c (h w) -> p c h w", w=WP))
                        z = zp.tile([P, nch * CF], BF16, name="z", tag="z")
                        nc.scalar.activation(
                            z.rearrange("p (c h w) -> p c h w", c=nch, w=W),
                            cvv[:, :, :, 1:1 + W],
                            AF.Relu, bias=ab[oc][:], scale=alpha_sc[oc][:],
                        )
                        ot = outp.tile([P, nch * CF], BF16, name="ot", tag="ot")
                        nc.vector.tensor_mul(
                            ot[:], z[:],
                            abv[:, img, c0 * CF:(c0 + nch) * CF])
                        nc.sync.dma_start(
                            out_d.ap()[img, oc * P:(oc + 1) * P,
                                       c0 * CH_ROWS:(c0 + nch) * CH_ROWS, :],
                            ot.rearrange("p (r w) -> p r w", w=W),
                        )

            sign_img(0, slabs=True)
            for img in range(NL):
                nxt = img + 1
                conv_img(img, (lambda i=nxt: sign_img(i)) if nxt < NL else None)

    nc.compile()
    return nc


_NC_CACHE: dict = {}


def _get_nc(n_local: int):
    if n_local not in _NC_CACHE:
        _NC_CACHE[n_local] = _build(n_local)
    return _NC_CACHE[n_local]


def _host_consts():
    ident = np.eye(P, dtype=np.float32).astype(NPBF16)
    tvt = np.zeros((HP, H), dtype=np.float32)
    for h in range(H):
        tvt[h:h + 3, h] = 1.0 / (9.0 * CIN)
    return ident, tvt.astype(NPBF16)


def _run(inputs: dict, trace: bool = False):
    x = np.asarray(inputs["x"], dtype=np.float32).astype(NPBF16)
    gamma = np.ascontiguousarray(np.asarray(inputs["gamma"], dtype=np.float32))
    beta_bn = np.ascontiguousarray(np.asarray(inputs["beta_bn"], dtype=np.float32))
    Wf = np.asarray(inputs["W"], dtype=np.float32).astype(NPBF16)
    # pure host-side layout permutation into the DoubleRow lhsT layout
    # [ki, tap, ko, co] with c = ko*128 + ki
    Wt = np.ascontiguousarray(
        Wf.reshape(COUT, 2, P, KTAPS).transpose(2, 3, 1, 0))
    b = np.ascontiguousarray(np.asarray(inputs["b"], dtype=np.float32))

    n = x.shape[0]
    assert n % NCORES == 0, f"batch {n} not divisible by {NCORES}"
    nl = n // NCORES
    nc = _get_nc(nl)
    ident, tvt = _host_consts()

    in_maps = []
    for i in range(NCORES):
        in_maps.append({
            "x": np.ascontiguousarray(x[i * nl:(i + 1) * nl]),
            "gamma": gamma, "beta_bn": beta_bn, "Wt": Wt, "b": b,
            "ident": ident, "tvt": tvt,
        })
    res = run_bass_kernel_spmd(nc, in_maps, core_ids=list(range(NCORES)),
                               trace=trace)
    out = np.concatenate(
        [res.results[i]["out"].astype(np.float32) for i in range(NCORES)],
        axis=0)
    return out, res


def kernel(**inputs) -> np.ndarray:
    out, _ = _run(inputs, trace=False)
    return out


def kernel_timed(**inputs):
    out, res = _run(inputs, trace=True)
    return out, res


# revision 10
# speedup vs baseline: 1.0726x; 1.0726x over previous
"""BinConv2d (XNOR-Net style) Trainium2 kernel, 8-core data-parallel, v4.

Layer math (BatchNorm train-mode -> BinActiv -> binary 3x3 conv -> scale by
box-filtered channel-mean magnitudes and per-filter alpha -> relu):

  mu, var: batch stats of x over (N, H, W) per channel      (needs all-reduce)
  xn  = (x - mu) * rsqrt(var + eps) * gamma + beta
  m   = mean_c |xn|;  xb = sign(xn);  Wb = sign(W);  alpha = mean |W| per filter
  y   = conv(xb, Wb, pad=1) + b
  out = relu(y * box3x3(m) * alpha)

v4 structure. The stats-landing time is floored by ncfw one-time setup
(~52us from the warmup trigger) + warmup mesh + real mesh ~= 72-76us, so the
only things that matter are (a) everything that does NOT depend on global
stats must be done by then, and (b) the land->conv-start chain must be
minimal.  v3 lost ~36us here: its ACT queue was still draining m-path ABS
work when the stats landed, and the global scalar chain was stuck behind
m-path copies in the DVE queue, so the first conv matmul issued ~36us after
the collective.

  pass 1:  load x (bf16, host-converted) into resident SBUF.  ALL BN stats
           on DVE bn_stats (ACT freed entirely); trigger lands ~45us which
           is still well before ncfw is ready, so it costs nothing.
  window:  weights arrive host-pre-transposed into the DoubleRow lhsT layout
           [ki, tap, ko, co] (pure host permutation), so weight prep is ONE
           ACT sign -> fp8 and ONE ACT abs -> fp8 (alpha via 36 tiny N=1
           PE matmuls against ones), no PE transposes, no per-tile signs.
           m path with LOCAL per-core stats as in v3 (ACT abs -> fp8 |xn|,
           ones-matmul per row chunk, box3x3, gpsimd broadcast).  ACT order:
           abs(kc0 imgs), weight sign/abs, abs(kc1 imgs) -> ACT idle from
           ~61us, waiting on the collective.
  land:    readback -> global scalar chain (DVE ops emitted after all
           window DVE work so they run immediately) -> sign img0 split in
           row slabs so conv group A starts after the first two slabs.
           Two PE warm-up chains (gated on the readback / on t') keep the
           PE HAM-hot through the land window so conv issues at 2.4 GHz.
  pass 2:  conv via 9 shifted DoubleRow matmuls per chunk group (PSUM 4+3
           banks), fused relu(scale,bias) on ACT, abeta multiply on DVE,
           bf16 DMA out.  Image 3 ends with a 1-chunk group to cut the tail.

  sign needs *global* mu (local-stats threshold flips ~0.24% of pixels =>
  ~10% output error), so the conv cannot start before the collective lands;
  the m path tolerates local stats.
"""

import os
import sys

import numpy as np

for _p in ("/opt/trn_rl_repo", "/root/.axon_site/_ro/trn_rl_repo"):
    if os.path.isdir(_p) and _p not in sys.path:
        sys.path.insert(0, _p)

import concourse.bass as bass  # noqa: E402
import concourse.bacc as bacc  # noqa: E402
import concourse.mybir as mybir  # noqa: E402
import concourse.tile as tile  # noqa: E402
from concourse.bass_utils import run_bass_kernel_spmd  # noqa: E402

F32 = mybir.dt.float32
BF16 = mybir.dt.bfloat16
FP8 = mybir.dt.float8e4
NPBF16 = mybir.dt.np(BF16)
AF = mybir.ActivationFunctionType
ALU = mybir.AluOpType
AX = mybir.AxisListType

EPS = 1e-4
NCORES = 8
P = 128
CIN = 256
COUT = 256
H = 56
W = 56
HP = H + 2          # 58 padded rows
WP = W + 2          # 58 padded cols
IMGP = HP * WP      # 3364 padded pixels / image
NPIX = H * W        # 3136 true pixels / image
MARGIN = 64         # dead zero margin absorbing out-of-image tap reads
CH_ROWS = 8         # output rows per PSUM bank
NCH = H // CH_ROWS  # 7 chunks / image
CF = CH_ROWS * W    # 448 compact free elems / chunk
CFP = CH_ROWS * WP  # 464 padded free elems / chunk
BANK = 512          # f32 elems per PSUM bank
KTAPS = 9
WARMUP = os.environ.get("BC_WARMUP", "1") == "1"


def _build(n_local: int):
    NL = n_local

    nc = bacc.Bacc("TRN2", debug=False, target_bir_lowering=False,
                   num_devices=NCORES)
    x_d = nc.declare_dram_parameter("x", [NL, CIN, H, W], BF16, isOutput=False)
    g_d = nc.declare_dram_parameter("gamma", [CIN], F32, isOutput=False)
    bb_d = nc.declare_dram_parameter("beta_bn", [CIN], F32, isOutput=False)
    # host-pre-permuted weights: [ki, tap, ko, co] with c = ko*128 + ki
    wt_d = nc.declare_dram_parameter("Wt", [P, KTAPS, 2, COUT], BF16,
                                     isOutput=False)
    b_d = nc.declare_dram_parameter("b", [COUT], F32, isOutput=False)
    id_d = nc.declare_dram_parameter("ident", [P, P], BF16, isOutput=False)
    tv_d = nc.declare_dram_parameter("tvt", [HP, H], BF16, isOutput=False)
    out_d = nc.declare_dram_parameter("out", [NL, COUT, H, W], BF16, isOutput=True)

    with tile.TileContext(nc, num_cores=NCORES) as tc:
        with (
            tc.tile_pool(name="statics", bufs=1) as st,
            tc.tile_pool(name="axp", bufs=3) as axp,
            tc.tile_pool(name="smalls", bufs=2) as sm,
            tc.tile_pool(name="zp", bufs=2) as zp,
            tc.tile_pool(name="outp", bufs=3) as outp,
            tc.tile_pool(name="psA", bufs=1, space="PSUM") as psA,
            tc.tile_pool(name="psB", bufs=1, space="PSUM") as psB,
            tc.tile_pool(name="psS", bufs=1, space="PSUM") as psS,
            tc.tile_pool(name="dram", bufs=1, space="DRAM") as dr,
        ):
            # ---------------- warmup collective (very first gpsimd op) -----
            # ncfw pays ~52us of one-time setup on the first collective; fire
            # a throwaway AllGather with no data deps immediately so the real
            # one only pays mesh latency.
            if WARMUP:
                wu_in = dr.tile([1, 8], F32, name="wu_in", tag="wu_in")
                wu_out = dr.tile([NCORES, 1, 8], F32, name="wu_out",
                                 tag="wu_out", addr_space="Shared")
                nc.gpsimd.collective_compute(
                    "AllGather", ALU.bypass,
                    replica_groups=[list(range(NCORES))],
                    ins=[wu_in.opt()], outs=[wu_out.opt()],
                )

            # ---------------- pass 1: load x + BN statistics (all DVE) -----
            xr = st.tile([P, 2 * NL * NPIX], BF16, name="xr", tag="xr")
            xrv = xr.rearrange("p (k i f) -> p k i f", k=2, i=NL)
            stats = []
            for kc in range(2):
                sb = st.tile([P, NL * NCH * 6], F32, name=f"stats{kc}",
                             tag=f"stats{kc}")
                stats.append(sb)
            for img in range(NL):
                for kc in range(2):
                    deng = nc.sync if kc == 0 else nc.scalar
                    deng.dma_start(
                        xrv[:, kc, img, :],
                        x_d.ap()[img, kc * P:(kc + 1) * P]
                        .rearrange("c h w -> c (h w)"),
                    )
            mean_h, var_h = [None, None], [None, None]
            pay = st.tile([P, 4], F32, name="pay", tag="pay")
            epsc = st.tile([P, 1], F32, name="epsc", tag="epsc")
            nc.vector.memset(epsc[:], EPS)
            ones2 = st.tile([P, 2], FP8, name="ones2", tag="ones2")
            nc.vector.memset(ones2[:], 1.0)

            # host constants + weights (scalar=HWDGE ring shared with kc1)
            ident = st.tile([P, P], BF16, name="ident_sb", tag="ident_sb")
            nc.scalar.dma_start(ident[:], id_d.ap())
            tvt = st.tile([HP, H], BF16, name="tvt_sb", tag="tvt_sb")
            nc.scalar.dma_start(tvt[:], tv_d.ap())
            gam2 = st.tile([P, 2], F32, name="gam2", tag="gam2")
            nc.scalar.dma_start(gam2[:], g_d.ap().rearrange("(k p) -> p k", k=2))
            bet2 = st.tile([P, 2], F32, name="bet2", tag="bet2")
            nc.scalar.dma_start(bet2[:], bb_d.ap().rearrange("(k p) -> p k", k=2))
            bvec2 = st.tile([P, 2], F32, name="bvec2", tag="bvec2")
            nc.scalar.dma_start(bvec2[:], b_d.ap().rearrange("(k p) -> p k", k=2))
            w_t = st.tile([P, KTAPS * 2 * COUT], BF16, name="w_t", tag="w_t")
            nc.sync.dma_start(w_t[:],
                              wt_d.ap().rearrange("p t k o -> p (t k o)"))

            ACT_STATS = []
            acc_s, acc_q = {}, {}

            s_loc, bstar = [None, None], [None, None]

            def kc_stats(kc):
                dve_imgs = [i for i in range(NL) if (kc, i) not in ACT_STATS]
                for gi, img in enumerate(dve_imgs):
                    for g in range(NCH):
                        col = (gi * NCH + g) * 6
                        nc.vector.bn_stats(
                            stats[kc][:, col:col + 6],
                            xrv[:, kc, img, g * CF:(g + 1) * CF],
                        )
                nd = len(dve_imgs)
                agg = st.tile([P, 2], F32, name=f"agg{kc}", tag=f"agg{kc}")
                nc.vector.bn_aggr(agg[:], stats[kc][:, 0:nd * NCH * 6])
                msq = sm.tile([P, 1], F32, name="msq", tag="msq")
                nc.vector.tensor_mul(msq[:], agg[:, 0:1], agg[:, 0:1])
                ex2 = st.tile([P, 1], F32, name=f"ex2_{kc}", tag=f"ex2_{kc}")
                nc.vector.tensor_add(ex2[:], agg[:, 1:2], msq[:])
                if nd == NL:
                    nc.vector.tensor_copy(pay[:, 2 * kc:2 * kc + 1], agg[:, 0:1])
                    nc.vector.tensor_copy(pay[:, 2 * kc + 1:2 * kc + 2], ex2[:])
                    mean_h[kc] = agg[:, 0:1]
                    var_h[kc] = agg[:, 1:2]
                else:
                    # combine DVE aggregate (nd imgs) with ACT raw sums
                    # (equal image weights): S = nd*NPIX*mean + sum(s_i)
                    a_imgs = [i for i in range(NL) if (kc, i) in ACT_STATS]
                    Ssum = sm.tile([P, 1], F32, name="Ssum", tag="Ssum")
                    nc.vector.tensor_add(Ssum[:], acc_s[(kc, a_imgs[0])][:],
                                         acc_s[(kc, a_imgs[1])][:])
                    mn = sm.tile([P, 1], F32, name="mn", tag="mn")
                    nc.vector.tensor_scalar_mul(mn[:], agg[:, 0:1],
                                                float(nd * NPIX))
                    nc.vector.tensor_add(Ssum[:], Ssum[:], mn[:])
                    Qsum = sm.tile([P, 1], F32, name="Qsum", tag="Qsum")
                    nc.vector.tensor_add(Qsum[:], acc_q[(kc, a_imgs[0])][:],
                                         acc_q[(kc, a_imgs[1])][:])
                    en = sm.tile([P, 1], F32, name="en", tag="en")
                    nc.vector.tensor_scalar_mul(en[:], ex2[:],
                                                float(nd * NPIX))
                    nc.vector.tensor_add(Qsum[:], Qsum[:], en[:])
                    meanf = st.tile([P, 1], F32, name=f"meanf{kc}",
                                    tag=f"meanf{kc}")
                    nc.vector.tensor_scalar_mul(meanf[:], Ssum[:],
                                                1.0 / (NL * NPIX))
                    ex2f = st.tile([P, 1], F32, name=f"ex2f{kc}",
                                   tag=f"ex2f{kc}")
                    nc.vector.tensor_scalar_mul(ex2f[:], Qsum[:],
                                                1.0 / (NL * NPIX))
                    nc.vector.tensor_copy(pay[:, 2 * kc:2 * kc + 1], meanf[:])
                    nc.vector.tensor_copy(pay[:, 2 * kc + 1:2 * kc + 2],
                                          ex2f[:])
                    mnsq = sm.tile([P, 1], F32, name="mnsq", tag="mnsq")
                    nc.vector.tensor_mul(mnsq[:], meanf[:], meanf[:])
                    varf = st.tile([P, 1], F32, name=f"varf{kc}",
                                   tag=f"varf{kc}")
                    nc.vector.tensor_sub(varf[:], ex2f[:], mnsq[:])
                    mean_h[kc] = meanf[:]
                    var_h[kc] = varf[:]
                # local scalars for the m path
                sigl = sm.tile([P, 1], F32, name="sigl", tag="sigl")
                nc.scalar.activation(sigl[:], var_h[kc], AF.Sqrt, bias=epsc[:])
                rsigl = sm.tile([P, 1], F32, name="rsigl", tag="rsigl")
                nc.vector.reciprocal(rsigl[:], sigl[:])
                sl = st.tile([P, 1], F32, name=f"sl{kc}", tag=f"sl{kc}")
                nc.vector.tensor_mul(sl[:], gam2[:, kc:kc + 1], rsigl[:])
                s_loc[kc] = sl
                smu = sm.tile([P, 1], F32, name="smu", tag="smu")
                nc.vector.tensor_mul(smu[:], sl[:], mean_h[kc])
                bs = st.tile([P, 1], F32, name=f"bs{kc}", tag=f"bs{kc}")
                nc.vector.tensor_sub(bs[:], bet2[:, kc:kc + 1], smu[:])
                bstar[kc] = bs

            kc_stats(0)

            # ---------------- window: m-path ABS kc0 (ACT, right after the
            # kc0 local scalars; kc1 half comes after the weight prep) ------
            ax_t = []
            for img in range(NL):
                ax = axp.tile([P, 2 * NPIX], FP8, name="ax", tag="ax")
                ax_t.append(ax.rearrange("p (k f) -> p k f", k=2))
                nc.scalar.activation(ax_t[img][:, 0, :], xrv[:, 0, img, :],
                                     AF.Abs, bias=bstar[0][:],
                                     scale=s_loc[0][:])

            # ---------------- window: weight prep (single sign + abs) ------
            wq = st.tile([P, KTAPS * 2 * COUT], FP8, name="wq", tag="wq")
            wqv = wq.rearrange("p (t k o) -> p t k o", t=KTAPS, k=2)
            nc.scalar.activation(wq[:], w_t[:], AF.Sign)
            awq = st.tile([P, KTAPS * 2 * COUT], FP8, name="awq", tag="awq")
            awqv = awq.rearrange("p (t k o) -> p t k o", t=KTAPS, k=2)
            nc.scalar.activation(awq[:], w_t[:], AF.Abs)
            # alpha[co] = mean |W[co,:,:,:]| : accumulate 18 N=1 matmuls
            aps_t = []
            for oc in range(2):
                aps = psS.tile([P, 1], F32, name="aps", tag="s")
                first = True
                for tap in range(KTAPS):
                    for ko in range(2):
                        nc.tensor.matmul(
                            aps[:], awqv[:, tap, ko, oc * P:(oc + 1) * P],
                            ones2[:, 0:1],
                            start=first, stop=(tap == KTAPS - 1 and ko == 1),
                        )
                        first = False
                aps_t.append(aps)

            # ---------------- pass 1b: kc1 stats + trigger -----------------
            kc_stats(1)
            cc_in = dr.tile([P, 4], F32, name="cc_in", tag="cc_in")
            cc_out = dr.tile([NCORES, P, 4], F32, name="cc_out", tag="cc_out",
                             addr_space="Shared")
            nc.sync.dma_start(cc_in[:], pay[:])
            nc.gpsimd.collective_compute(
                "AllGather", ALU.bypass,
                replica_groups=[list(range(NCORES))],
                ins=[cc_in.opt()], outs=[cc_out.opt()],
            )

            # ---------------- static pads (gpsimd) -------------------------
            # xq blocks: [P, img, ko, M | IMGP | M] so the DoubleRow rhs view
            # (k-stride = BLK) bounding-boxes only ONE image's two halves.
            BLK = IMGP + 2 * MARGIN
            xq = st.tile([P, NL * 2 * BLK], FP8, name="xq", tag="xq")
            xqb = xq.rearrange("p (i k b) -> p i k b", i=NL, k=2)
            for img in range(NL):
                for ko in range(2):
                    nc.gpsimd.memset(xqb[:, img, ko, 0:MARGIN + WP], 0.0)
                    nc.gpsimd.memset(
                        xqb[:, img, ko, MARGIN + (HP - 1) * WP:BLK], 0.0)
                    colv = (xqb[:, img, ko, MARGIN + WP: MARGIN + (HP - 1) * WP]
                            .rearrange("p (h w) -> p h w", w=WP))
                    nc.gpsimd.memset(colv[:, :, 0:1], 0.0)
                    nc.gpsimd.memset(colv[:, :, WP - 1:WP], 0.0)
            m_flat = st.tile([1, NL * IMGP], BF16, name="m_flat", tag="m_flat")
            mfl = m_flat.rearrange("p (i f) -> p i f", i=NL)
            nc.gpsimd.memset(mfl[:, :, 0:WP], 0.0)
            nc.gpsimd.memset(mfl[:, :, (HP - 1) * WP:IMGP], 0.0)
            mfv = mfl[:, :, WP:(HP - 1) * WP].rearrange("p i (h w) -> p i h w",
                                                        w=WP)
            nc.gpsimd.memset(mfv[:, :, :, 0:1], 0.0)
            nc.gpsimd.memset(mfv[:, :, :, WP - 1:WP], 0.0)

            # ---------------- window: m path (kc1 abs + chunk matmuls) -----
            abeta = st.tile([P, NL * NPIX], BF16, name="abeta", tag="abeta")
            abv = abeta.rearrange("p (i f) -> p i f", i=NL)
            for img in range(NL):
                nc.scalar.activation(ax_t[img][:, 1, :], xrv[:, 1, img, :],
                                     AF.Abs, bias=bstar[1][:],
                                     scale=s_loc[1][:])
                for ch in range(NCH):
                    # ping-pong between psS and the (window-idle) conv psB
                    # bank so the per-chunk mm->copy chain pipelines
                    pool, ptag = ((psS, "s") if (img * NCH + ch) % 2 == 0
                                  else (psB, "cvB"))
                    mp = pool.tile([1, CF], F32, name="mp", tag=ptag)
                    nc.tensor.matmul(mp[:], ones2[:, 0:1],
                                     ax_t[img][:, 0, ch * CF:(ch + 1) * CF],
                                     start=True, stop=False)
                    nc.tensor.matmul(mp[:], ones2[:, 1:2],
                                     ax_t[img][:, 1, ch * CF:(ch + 1) * CF],
                                     start=False, stop=True)
                    mfi = (m_flat[:, img * IMGP:(img + 1) * IMGP]
                           .rearrange("p (h w) -> p h w", w=WP))
                    nc.vector.tensor_copy(
                        mfi[:, 1 + ch * CH_ROWS: 1 + (ch + 1) * CH_ROWS,
                            1:1 + W],
                        mp.rearrange("p (h w) -> p h w", w=W),
                    )
                # beta map: horizontal sum on DVE, vertical via banded matmul
                mhw = sm.tile([HP, WP], BF16, name="mhw", tag="mhw")
                nc.sync.dma_start(mhw[:], m_flat[:, img * IMGP:(img + 1) * IMGP])
                hs = sm.tile([HP, WP], BF16, name="hs", tag="hs")
                nc.vector.tensor_add(hs[:, 1:1 + W], mhw[:, 0:W], mhw[:, 2:2 + W])
                nc.vector.tensor_add(hs[:, 1:1 + W], hs[:, 1:1 + W],
                                     mhw[:, 1:1 + W])
                bps = psS.tile([H, W], F32, name="bps", tag="s")
                nc.tensor.matmul(bps[:], tvt[:], hs[:, 1:1 + W], start=True,
                                 stop=True)
                bhw = sm.tile([H, W], BF16, name="bhw", tag="bhw")
                nc.vector.tensor_copy(bhw[:], bps[:])
                bflat = sm.tile([1, NPIX], BF16, name="bflat", tag="bflat",
                                bufs=2)
                nc.sync.dma_start(bflat[:], bhw[:])
                nc.gpsimd.partition_broadcast(abv[:, img, :], bflat[:])

            # ---------------- global stats readback + scalars --------------
            # readback on the scalar (ACT) ring: ACT is idle from ~65us on,
            # so head-blocking its queue on the collective sem is free, and
            # the sync ring keeps flowing for the m-path + output stores.
            ag_sb = st.tile([P, NCORES * 4], F32, name="ag_sb", tag="ag_sb")
            nc.scalar.dma_start(
                ag_sb[:].rearrange("p (r c) -> p r c", c=4),
                cc_out.rearrange("r p c -> p r c"),
            )
            # alpha scalars on DVE (emitted late so they never gate the
            # stats payload in the DVE queue; needed only by the first relu)
            alpha_sc, ab = [], []
            for oc in range(2):
                asc = st.tile([P, 1], F32, name=f"alph{oc}", tag=f"alph{oc}")
                nc.vector.tensor_scalar_mul(asc[:], aps_t[oc][:],
                                            1.0 / (CIN * KTAPS))
                alpha_sc.append(asc)
                abt = st.tile([P, 1], F32, name=f"ab{oc}", tag=f"ab{oc}")
                nc.vector.tensor_mul(abt[:], asc[:], bvec2[:, oc:oc + 1])
                ab.append(abt)
            rgam = st.tile([P, 2], F32, name="rgam", tag="rgam")
            nc.vector.reciprocal(rgam[:], gam2[:])
            brg = st.tile([P, 2], F32, name="brg", tag="brg")
            nc.vector.tensor_mul(brg[:], bet2[:], rgam[:])
            arsb = st.tile([P, 4], F32, name="arsb", tag="arsb")
            nc.vector.tensor_reduce(
                arsb[:], ag_sb[:].rearrange("p (r c) -> p c r", c=4),
                axis=AX.X, op=ALU.add,
            )
            mue = st.tile([P, 4], F32, name="mue", tag="mue")
            nc.vector.tensor_scalar_mul(mue[:], arsb[:], 1.0 / NCORES)
            muev = mue.rearrange("p (c k) -> p c k", c=2)  # [P, kc, (mean,ex2)]
            muv = muev[:, :, 0]
            musq = sm.tile([P, 2], F32, name="musq2", tag="musq2")
            nc.vector.tensor_mul(musq[:], muv, muv)
            varv = sm.tile([P, 2], F32, name="varv", tag="varv")
            nc.vector.tensor_sub(varv[:], muev[:, :, 1], musq[:])
            # PE p-state warmup: one short chain gated on the collective
            # result (first MM reads wrm; the rest chain via the psS WAR
            # dependency), dense enough to unthrottle HAM before the conv.
            # v4's 148-MM chains got interleaved into the conv by the
            # scheduler and cost ~15us of serialized queue time.
            wrm = sm.tile([P, 4], BF16, name="wrm", tag="wrm")
            nc.vector.tensor_copy(wrm[:], arsb[:])
            pd = psS.tile([P, 4], F32, name="pd", tag="s")
            nc.tensor.matmul(pd[:], ident[:], wrm[:], start=True, stop=True)
            for _ in range(8):
                pd = psS.tile([P, CF], F32, name="pd", tag="s")
                nc.tensor.matmul(pd[:], ident[:], xr[:, 0:CF], start=True,
                                 stop=True)
            sigv = sm.tile([P, 2], F32, name="sigv", tag="sigv")
            nc.scalar.activation(sigv[:], varv[:], AF.Sqrt, bias=epsc[:])
            tb2 = sm.tile([P, 2], F32, name="tb2", tag="tb2")
            nc.vector.tensor_mul(tb2[:], brg[:], sigv[:])
            tp = st.tile([P, 2], F32, name="tp", tag="tp")
            nc.vector.tensor_sub(tp[:], tb2[:], muv)
            # ---------------- pass 2: sign + conv + epilogue ----------------
            GRPS_STD = [(0, 4, psA, "cvA"), (4, 3, psB, "cvB")]
            GRPS_LAST = [(0, 3, psA, "cvA"), (3, 3, psB, "cvB"),
                         (6, 1, psS, "s")]

            def sign_img(img, slabs=False):
                # slabs: kc0 whole, then kc1 rows 0..33 (the DoubleRow rhs
                # bounding box spans the whole kc0 block, so splitting kc0
                # buys nothing; group A of the first image reads kc1 padded
                # rows <= 34), so conv starts ~1us earlier.
                rr = ([(0, 0, H), (1, 0, 34), (1, 34, H)] if slabs
                      else [(0, 0, H), (1, 0, H)])
                for kc, r0, r1 in rr:
                    xqv = (xqb[:, img, kc, MARGIN:MARGIN + IMGP]
                           .rearrange("p (h w) -> p h w", w=WP))
                    nc.scalar.activation(
                        xqv[:, 1 + r0:1 + r1, 1:1 + W],
                        xrv[:, kc, img, r0 * W:r1 * W]
                        .rearrange("p (h w) -> p h w", w=W),
                        AF.Sign, bias=tp[:, kc:kc + 1],
                    )

            def conv_img(img, mid_cb=None):
                grps = GRPS_LAST if img == NL - 1 else GRPS_STD
                for oc in range(2):
                    if oc == 1 and mid_cb is not None:
                        # queue the next image's signs on ACT *between* this
                        # image's oc halves: the oc0 relus stay ahead of the
                        # signs so the oc1 PSUM reuse never stalls on ACT
                        mid_cb()
                    for (c0, nch, pool, tag) in grps:
                        cv = pool.tile([P, nch * BANK], F32, name="cv", tag=tag)
                        for tap in range(KTAPS):
                            dh, dw = tap // 3, tap % 3
                            off = (dh - 1) * WP + (dw - 1)
                            for ch in range(nch):
                                base = (MARGIN
                                        + (1 + (c0 + ch) * CH_ROWS) * WP + off)
                                nc.tensor.matmul(
                                    cv[:, ch * BANK:ch * BANK + CFP],
                                    wqv[:, tap, :, oc * P:(oc + 1) * P],
                                    xqb[:, img, :, base: base + CFP],
                                    start=(tap == 0), stop=(tap == KTAPS - 1),
                                    perf_mode=mybir.MatmulPerfMode.DoubleRow,
                                )
                        # fused relu(alpha*cv + alpha*b) over the whole group
                        cvv = (cv.rearrange("p (c x) -> p c x", x=BANK)
                               [:, :, 0:CFP]
                               .rearrange("p c (h w) -> p c h w", w=WP))
                        z = zp.tile([P, nch * CF], BF16, name="z", tag="z")
                        nc.scalar.activation(
                            z.rearrange("p (c h w) -> p c h w", c=nch, w=W),
                            cvv[:, :, :, 1:1 + W],
                            AF.Relu, bias=ab[oc][:], scale=alpha_sc[oc][:],
                        )
                        ot = outp.tile([P, nch * CF], BF16, name="ot", tag="ot")
                        nc.vector.tensor_mul(
                            ot[:], z[:],
                            abv[:, img, c0 * CF:(c0 + nch) * CF])
                        nc.sync.dma_start(
                            out_d.ap()[img, oc * P:(oc + 1) * P,
                                       c0 * CH_ROWS:(c0 + nch) * CH_ROWS, :],
                            ot.rearrange("p (r w) -> p r w", w=W),
                        )

            sign_img(0, slabs=True)
            for img in range(NL):
                nxt = img + 1
                conv_img(img, (lambda i=nxt: sign_img(i)) if nxt < NL else None)

    nc.compile()
    return nc


_NC_CACHE: dict = {}


def _get_nc(n_local: int):
    if n_local not in _NC_CACHE:
        _NC_CACHE[n_local] = _build(n_local)
    return _NC_CACHE[n_local]


def _host_consts():
    ident = np.eye(P, dtype=np.float32).astype(NPBF16)
    tvt = np.zeros((HP, H), dtype=np.float32)
    for h in range(H):
        tvt[h:h + 3, h] = 1.0 / (9.0 * CIN)
    return ident, tvt.astype(NPBF16)


def _run(inputs: dict, trace: bool = False):
    x = np.asarray(inputs["x"], dtype=np.float32).astype(NPBF16)
    gamma = np.ascontiguousarray(np.asarray(inputs["gamma"], dtype=np.float32))
    beta_bn = np.ascontiguousarray(np.asarray(inputs["beta_bn"], dtype=np.float32))
    Wf = np.asarray(inputs["W"], dtype=np.float32).astype(NPBF16)
    # pure host-side layout permutation into the DoubleRow lhsT layout
    # [ki, tap, ko, co] with c = ko*128 + ki
    Wt = np.ascontiguousarray(
        Wf.reshape(COUT, 2, P, KTAPS).transpose(2, 3, 1, 0))
    b = np.ascontiguousarray(np.asarray(inputs["b"], dtype=np.float32))

    n = x.shape[0]
    assert n % NCORES == 0, f"batch {n} not divisible by {NCORES}"
    nl = n // NCORES
    nc = _get_nc(nl)
    ident, tvt = _host_consts()

    in_maps = []
    for i in range(NCORES):
        in_maps.append({
            "x": np.ascontiguousarray(x[i * nl:(i + 1) * nl]),
            "gamma": gamma, "beta_bn": beta_bn, "Wt": Wt, "b": b,
            "ident": ident, "tvt": tvt,
        })
    res = run_bass_kernel_spmd(nc, in_maps, core_ids=list(range(NCORES)),
                               trace=trace)
    out = np.concatenate(
        [res.results[i]["out"].astype(np.float32) for i in range(NCORES)],
        axis=0)
    return out, res


def kernel(**inputs) -> np.ndarray:
    out, _ = _run(inputs, trace=False)
    return out


def kernel_timed(**inputs):
    out, res = _run(inputs, trace=True)
    return out, res
